# revision 11
# baseline (speedup 1.0000x reference)
"""Trainium2 Bass kernel for nn_DeterministicAdjacency (gnn_message_passing).

Math (reference):
    hi = z @ W1[:D]            # (K, E)
    hj = z @ W1[D:]            # (K, E)
    h  = silu(hi[:,None,:] + hj[None,:,:] + b1)    # (K, K, E)
    logits = einsum('ije,eo->ij', h, W2) + b2      # (K, K)
    out = softmax(logits, axis=-1)

b2 is dropped: softmax is invariant to a constant shift.

Sharding: rows (i / query dim) split across 8 cores, 256 rows each. Each core
computes its 256 rows of logits against the full z and does local row softmax.

Per-core layout ("layout A", e on partitions):
  - hjbT2 (128p=(s,e), 2048f=j): hj^T + b1, duplicated on both partition
    halves (s = row-parity slot), fp16. Computed in 4 j-chunks of 512 so the
    first silu can start while the zT DMA is still landing.
  - hibP (128p=(s,e), 128f=k): bias columns; column k holds
    [hi[2k,:] ; hi[2k+1,:]] so one ScalarE activation instruction computes
    silu for TWO query rows x all 2048 keys x all 64 features.
  - contraction over e via TensorE with a NARROW stationary: pairs are
    processed in blocks of 16; pair kk uses a (128 x 32) stationary slice
    (stat[(s,e), 2*(kk%16)+s] = W2[e]) and accumulates into the 32-aligned
    psum partition slice [32*(kk//16) : +32] of a (128, 4, 512) accumulator.
    Only 32 of 128 PE columns are active -> less PE/SBUF energy (this kernel
    is ACT-bound and the chip duty-throttles on power: ~0.8 avg util limit
    was observed with the dense 128-wide stationary layout).
  - h/xg/hjbT2 are fp16: DVE runs in 4x perf mode for the bias-add
    precompute, halving SBUF traffic; PE fp16 path is 1 cyc/row.
  - steady state: DVE precomputes x = hjbT2 + bias for groups of 8 pairs,
    then ONE 16384-wide ScalarE silu amortizes the per-instruction bubble.
  - softmax fused on the PSUM accumulators (ACT exp, DVE row-sum +
    reciprocal + scale; logits are O(+-6) so max-subtraction is skipped),
    then chunked DMA out. Row sums use DVE tensor_reduce instead of the ACT
    accumulator: READ_ACCUMULATOR forces a serializing engine drain between
    the two exp instructions.
"""

import numpy as np

import concourse.bass as bass
import concourse.bacc as bacc
import concourse.mybir as mybir
from concourse import tile
from concourse.bass_utils import run_bass_kernel_spmd

K, D, E = 2048, 128, 64
NCORES = 8
R = K // NCORES            # 256 rows per core
NPAIR = 64                 # row pairs per 128-row i-tile
NT = 4                     # 512-wide j tiles
WARM = 2                   # chunked warm-up pairs
G = 8                      # steady-state group size
TAIL = 4                   # per-pair tail pairs
NDVE = 1                   # pairs per full group offloaded to the DVE silu
F32 = mybir.dt.float32
F16 = mybir.dt.float16
AF = mybir.ActivationFunctionType
AX = mybir.AxisListType
ALU = mybir.AluOpType


def build_nc() -> bass.Bass:
    # Bacc (not raw Bass): its finalize() runs generate_event_semaphores(),
    # which splits multi-sem waits — TRN2 instructions hold at most one wait.
    nc = bacc.Bacc(None, target_bir_lowering=False)
    # zTc comes in fp16, pre-transposed and pre-chunked (host layout prep):
    # contiguous 128KB DMAs, d already on partitions for the hj contraction.
    zTc_d = nc.declare_dram_parameter("zTc", [NT, D, 512], F16, isOutput=False)
    zcT_d = nc.declare_dram_parameter("zcT", [D, R], F16, isOutput=False)
    # w1a2/w1b2 = [W1a | W1a], [W1b | W1b]: one matmul emits both
    # partition-halves of the (s,e)-duplicated layouts directly.
    w1a2 = nc.declare_dram_parameter("w1a2", [D, 128], F16, isOutput=False)
    w1b2 = nc.declare_dram_parameter("w1b2", [D, 128], F16, isOutput=False)
    b1c2 = nc.declare_dram_parameter("b1c2", [128, 1], F32, isOutput=False)
    # 32 distinct narrow stationary slices (pair kk uses slice kk%32)
    stat_d = nc.declare_dram_parameter("stat", [128, 32, 64], F16, isOutput=False)
    out = nc.declare_dram_parameter("out", [R, K], F32, isOutput=True)

    with tile.TileContext(nc) as tc:
        with tc.tile_pool(name="singles", bufs=1) as singles:
            w1a_sb = singles.tile([D, 128], F16)
            w1b_sb = singles.tile([D, 128], F16)
            b1_sb = singles.tile([128, 1], F32)
            stat_sb = singles.tile([128, 32, 64], F16)
            zT = singles.tile([128, NT, 512], F16)
            zcT = singles.tile([128, R], F16)
            hjbT2 = singles.tile([128, K], F16)
            hjbT2h = singles.tile([128, K], F16)   # 0.5 * (hj + b1)
            hibP = singles.tile([128, 2 * NPAIR], F32)
            hibPh = singles.tile([128, 2 * NPAIR], F32)

            # hibP inputs first (ph matmul gates the first silu's bias), then
            # zT chunk 0 (gates the first silu), stat (needed by the first
            # matmul), then the rest of zT.
            nc.sync.dma_start(out=w1a_sb[:], in_=w1a2[:])
            nc.sync.dma_start(out=zcT[:], in_=zcT_d[:])
            nc.sync.dma_start(out=w1b_sb[:], in_=w1b2[:])
            nc.sync.dma_start(out=b1_sb[:], in_=b1c2[:])
            nc.sync.dma_start(out=zT[:, 0, :], in_=zTc_d[0, :, :])
            nc.sync.dma_start(out=stat_sb[:], in_=stat_d[:])
            for c in range(1, NT):
                nc.sync.dma_start(out=zT[:, c, :], in_=zTc_d[c, :, :])

            with tc.tile_pool(name="acc0p", bufs=1, space="PSUM") as acc0p:
                acc0 = acc0p.tile([128, NT, 512], F32, name="acc0")

                # ---- prologue: hi / hj projections (chunked) ----
                with tc.tile_pool(name="pp", bufs=1, space="PSUM") as pp:
                    # hiT (both halves) -> pair-bias columns; lane-aligned
                    # copies (even cols land on s=0 half, odd on s=1).
                    ph = pp.tile([128, R], F32, tag="ph")
                    nc.tensor.matmul(ph[:], w1a_sb[:], zcT[:], start=True, stop=True)
                    phr = ph.rearrange("e (k two) -> e two k", two=2)
                    nc.vector.tensor_copy(hibP[0:E, :], phr[0:E, 0, :])
                    nc.vector.tensor_copy(hibP[E:128, :], phr[E:128, 1, :])
                    nc.vector.tensor_scalar_mul(
                        out=hibPh[:], in0=hibP[:], scalar1=0.5
                    )

                    for t in range(NT):
                        # hjT + b1, both (s,e) halves at once via [W1b|W1b].
                        pj = pp.tile([128, 512], F32, tag="pj", bufs=2)
                        sl = slice(t * 512, (t + 1) * 512)
                        nc.tensor.matmul(pj[:], w1b_sb[:], zT[:, t, :],
                                         start=True, stop=True)
                        nc.vector.tensor_scalar_add(
                            out=hjbT2[:, sl], in0=pj[:], scalar1=b1_sb[:],
                        )
                        nc.vector.tensor_scalar(
                            out=hjbT2h[:, sl], in0=pj[:], scalar1=b1_sb[:],
                            scalar2=0.5, op0=ALU.add, op1=ALU.mult,
                        )

                # ---- main loop: silu + e-contraction into PSUM ----
                with (
                    tc.tile_pool(name="acc1p", bufs=1, space="PSUM") as acc1p,
                    tc.tile_pool(name="hp", bufs=1) as hp,
                    tc.tile_pool(name="dp", bufs=1) as dp,
                    tc.tile_pool(name="ep", bufs=1) as ep,
                    tc.tile_pool(name="sp", bufs=4) as sp,
                ):
                    acc1 = acc1p.tile([128, NT, 512], F32, name="acc1")
                    acc = {0: acc0, 1: acc1}

                    def dve_silu(k, out_ap):
                        """silu via DVE Pade: y + u*P(u)/Q(u), y=x/2, u=y^2.
                        tanh(y) ~ y*(945+105u+u^2)/(945+420u+15u^2); accurate
                        to ~3e-4 for |y|<3 (real |x| max is ~5.9). Offloads
                        pairs from the saturated ACT engine to DVE slack."""
                        yv = dp.tile([128, K], F16, tag="y")
                        uv = dp.tile([128, K], F16, tag="u")
                        A = dp.tile([128, K], F16, tag="A")
                        B = dp.tile([128, K], F16, tag="B")
                        nc.vector.tensor_scalar_add(
                            out=yv[:], in0=hjbT2h[:],
                            scalar1=hibPh[:, k : k + 1],
                        )
                        nc.vector.tensor_mul(uv[:], yv[:], yv[:])
                        nc.vector.scalar_tensor_tensor(
                            out=A[:], in0=uv[:], scalar=105.0, in1=uv[:],
                            op0=ALU.add, op1=ALU.mult,
                        )
                        nc.vector.scalar_tensor_tensor(
                            out=B[:], in0=A[:], scalar=945.0, in1=uv[:],
                            op0=ALU.add, op1=ALU.mult,
                        )
                        nc.vector.scalar_tensor_tensor(
                            out=A[:], in0=uv[:], scalar=28.0, in1=uv[:],
                            op0=ALU.add, op1=ALU.mult,
                        )
                        nc.vector.tensor_scalar(
                            out=A[:], in0=A[:], scalar1=15.0, scalar2=945.0,
                            op0=ALU.mult, op1=ALU.add,
                        )
                        with nc.allow_low_precision("fp16 silu approx"):
                            nc.vector.reciprocal(out=uv[:], in_=A[:])
                        nc.vector.tensor_mul(A[:], B[:], uv[:])
                        nc.vector.tensor_add(out_ap, yv[:], A[:])

                    def contract(k, h_ap, t):
                        """acc rows 2kk,2kk+1 (64-aligned block) += W2 @ h"""
                        u, kk = divmod(k, NPAIR)
                        b, slot = divmod(kk, 32)
                        nc.tensor.matmul(
                            acc[u][64 * b : 64 * b + 64, t, :],
                            stat_sb[:, slot, :],
                            h_ap,
                            start=(slot == 0),
                            stop=(slot == 31),
                        )

                    # Warm-up pairs, chunked per 512-wide j tile: silu starts
                    # as soon as hjbT2 chunk 0 + hibP are ready (zT chunk
                    # DMAs and the pj chain are still in flight).
                    for k in range(WARM):
                        h = hp.tile([128, K], F16, tag="h", bufs=2)
                        for c in range(NT):
                            sl = slice(c * 512, (c + 1) * 512)
                            nc.scalar.activation(
                                out=h[:, sl], in_=hjbT2[:, sl], func=AF.Silu,
                                bias=hibP[:, k : k + 1], scale=1.0,
                            )
                            contract(k, h[:, sl], c)

                    # Steady state: DVE precomputes x = hjbT2 + bias for a
                    # group of pairs (fp16, 4x perf mode), then ONE wide
                    # ScalarE silu covers the group. First group is small
                    # (it only needs to bridge until the pipeline fills).
                    k0 = WARM
                    for gsz in [2] + [G] * ((R // 2 - WARM - 2 - TAIL) // G):
                        # in full groups, the last NDVE pairs run on DVE
                        nact = gsz - (NDVE if gsz == G else 0)
                        xg = hp.tile([128, G, K], F16, tag="xg", bufs=2)
                        hg = hp.tile([128, G, K], F16, tag="hg", bufs=2)
                        for g in range(nact):
                            nc.vector.tensor_scalar_add(
                                out=xg[:, g, :], in0=hjbT2[:],
                                scalar1=hibP[:, k0 + g : k0 + g + 1],
                            )
                        nc.scalar.activation(
                            out=hg[:, 0:nact, :].rearrange("p g j -> p (g j)"),
                            in_=xg[:, 0:nact, :].rearrange("p g j -> p (g j)"),
                            func=AF.Silu,
                        )
                        for g in range(nact, gsz):
                            dve_silu(k0 + g, hg[:, g, :])
                        for g in range(gsz):
                            for t in range(NT):
                                contract(k0 + g, hg[:, g, t * 512 : (t + 1) * 512], t)
                        k0 += gsz

                    # last pairs go per-pair so the final MM+softmax chain
                    # after the last silu is short
                    for k in range(k0, R // 2):
                        h = hp.tile([128, K], F16, tag="h", bufs=2)
                        nc.scalar.activation(
                            out=h[:], in_=hjbT2[:], func=AF.Silu,
                            bias=hibP[:, k : k + 1], scale=1.0,
                        )
                        for t in range(NT):
                            contract(k, h[:, t * 512 : (t + 1) * 512], t)

                    # ---- fused row softmax + store ----
                    # logits are O(+-6) here, so exp without max-subtraction
                    # is safe in fp32 and drops the serial max chain.
                    for u in range(R // 128):
                        tot = sp.tile([128, 1], F32, tag="tot")
                        rec = sp.tile([128, 1], F32, tag="rec")
                        ex = ep.tile([128, K], F32, tag=f"ex{u}")
                        nc.scalar.activation(
                            out=ex.rearrange("p (t j) -> p t j", t=NT),
                            in_=acc[u][:], func=AF.Exp,
                        )
                        nc.vector.tensor_reduce(
                            out=tot[:], in_=ex[:], axis=AX.X, op=ALU.add,
                        )
                        nc.vector.reciprocal(out=rec[:], in_=tot[:])
                        # chunked normalize+store so the DMA overlaps scale
                        for c in range(2):
                            sl = slice(c * (K // 2), (c + 1) * (K // 2))
                            nc.vector.tensor_scalar_mul(
                                out=ex[:, sl], in0=ex[:, sl], scalar1=rec[:]
                            )
                            nc.sync.dma_start(
                                out=out[u * 128 : (u + 1) * 128, sl],
                                in_=ex[:, sl],
                            )
    nc.finalize()  # Bacc.compile(): wait splitting, reg alloc, act tables
    return nc


_CACHE: dict = {}


def _get_nc() -> bass.Bass:
    if "nc" not in _CACHE:
        _CACHE["nc"] = build_nc()
    return _CACHE["nc"]


def make_in_maps(z, W1, b1, W2):
    z = np.ascontiguousarray(np.asarray(z, np.float32))
    W1 = np.asarray(W1, np.float32)
    b1 = np.asarray(b1, np.float32)
    W2 = np.asarray(W2, np.float32)

    # narrow stationary: pair kk uses slice kk%32, columns 2*(kk%32)+s
    stat = np.zeros((128, 32, 64), np.float32)
    w2col = W2[:, 0]
    for slot in range(32):
        for s in range(2):
            stat[s * E : (s + 1) * E, slot, 2 * slot + s] = w2col
    stat = stat.astype(np.float16)
    b1c2 = np.ascontiguousarray(np.tile(b1, 2).reshape(128, 1))
    w1a2 = np.ascontiguousarray(np.tile(W1[:D], (1, 2)).astype(np.float16))
    w1b2 = np.ascontiguousarray(np.tile(W1[D:], (1, 2)).astype(np.float16))
    zT16 = np.ascontiguousarray(z.astype(np.float16).T)  # (D, K)
    zTc = np.ascontiguousarray(
        zT16.reshape(D, NT, 512).transpose(1, 0, 2)
    )  # (NT, D, 512)

    in_maps = []
    for c in range(NCORES):
        in_maps.append(
            {
                "zTc": zTc,
                "zcT": np.ascontiguousarray(zT16[:, c * R : (c + 1) * R]),
                "w1a2": w1a2,
                "w1b2": w1b2,
                "b1c2": b1c2,
                "stat": stat,
            }
        )
    return in_maps


def run(inputs: dict, trace: bool = False):
    """Run the bass kernel; returns (full_output, BassKernelResults)."""
    nc = _get_nc()
    in_maps = make_in_maps(inputs["z"], inputs["W1"], inputs["b1"], inputs["W2"])
    res = run_bass_kernel_spmd(nc, in_maps, list(range(NCORES)), trace=trace)
    full = np.concatenate([res.results[c]["out"] for c in range(NCORES)], axis=0)
    return full, res


def kernel(**inputs) -> np.ndarray:
    full, _ = run(inputs, trace=False)
    return full


# revision 16
# speedup vs baseline: 1.5413x; 1.5413x over previous
"""Trainium2 Bass kernel for nn_DeterministicAdjacency (gnn_message_passing).

Math (reference):
    hi = z @ W1[:D]            # (K, E)
    hj = z @ W1[D:]            # (K, E)
    h  = silu(hi[:,None,:] + hj[None,:,:] + b1)    # (K, K, E)
    logits = einsum('ije,eo->ij', h, W2) + b2      # (K, K)
    out = softmax(logits, axis=-1)

b2 is dropped: softmax is invariant to a constant shift.

Sharding: rows (i / query dim) split across 8 cores, 256 rows each. Each core
computes its 256 rows of logits against the full z and does local row softmax.

Per-core layout ("layout A", e on partitions):
  - hjbT2 (128p=(s,e), 2048f=j): hj^T + b1, duplicated on both partition
    halves (s = row-parity slot), fp16. Computed in 4 j-chunks of 512 so the
    first silu can start while the zT DMA is still landing.
  - hibP (128p=(s,e), 128f=k): bias columns; column k holds
    [hi[2k,:] ; hi[2k+1,:]] so one ScalarE activation instruction computes
    silu for TWO query rows x all 2048 keys x all 64 features.
  - contraction over e via TensorE with a NARROW stationary: pairs are
    processed in blocks of 16; pair kk uses a (128 x 32) stationary slice
    (stat[(s,e), 2*(kk%16)+s] = W2[e]) and accumulates into the 32-aligned
    psum partition slice [32*(kk//16) : +32] of a (128, 4, 512) accumulator.
    Only 32 of 128 PE columns are active -> less PE/SBUF energy (this kernel
    is ACT-bound and the chip duty-throttles on power: ~0.8 avg util limit
    was observed with the dense 128-wide stationary layout).
  - h/xg/hjbT2 are fp16: DVE runs in 4x perf mode for the bias-add
    precompute, halving SBUF traffic; PE fp16 path is 1 cyc/row.
  - steady state: DVE precomputes x = hjbT2 + bias for groups of 8 pairs,
    then ONE 16384-wide ScalarE silu amortizes the per-instruction bubble.
  - softmax fused on the PSUM accumulators (ACT exp, DVE row-sum +
    reciprocal + scale; logits are O(+-6) so max-subtraction is skipped),
    then chunked DMA out. Row sums use DVE tensor_reduce instead of the ACT
    accumulator: READ_ACCUMULATOR forces a serializing engine drain between
    the two exp instructions.
"""

import numpy as np

import concourse.bass as bass
import concourse.bacc as bacc
import concourse.mybir as mybir
from concourse import tile
from concourse.bass_utils import run_bass_kernel_spmd

K, D, E = 2048, 128, 64
NCORES = 8
R = K // NCORES            # 256 rows per core
NPAIR = 64                 # row pairs per 128-row i-tile
NT = 4                     # 512-wide j tiles
WARM = 2                   # chunked warm-up pairs
G = 8                      # steady-state group size
TAIL = 4                   # per-pair tail pairs
F32 = mybir.dt.float32
F16 = mybir.dt.float16
AF = mybir.ActivationFunctionType
AX = mybir.AxisListType
ALU = mybir.AluOpType


def build_nc() -> bass.Bass:
    # Bacc (not raw Bass): its finalize() runs generate_event_semaphores(),
    # which splits multi-sem waits — TRN2 instructions hold at most one wait.
    nc = bacc.Bacc(None, target_bir_lowering=False)
    # zTc comes in fp16, pre-transposed and pre-chunked (host layout prep):
    # contiguous 128KB DMAs, d already on partitions for the hj contraction.
    zTc_d = nc.declare_dram_parameter("zTc", [NT, D, 512], F16, isOutput=False)
    zcT_d = nc.declare_dram_parameter("zcT", [D, R], F16, isOutput=False)
    # w1a2/w1b2 = [W1a | W1a], [W1b | W1b]: one matmul emits both
    # partition-halves of the (s,e)-duplicated layouts directly.
    w1a2 = nc.declare_dram_parameter("w1a2", [D, 128], F16, isOutput=False)
    w1b2 = nc.declare_dram_parameter("w1b2", [D, 128], F16, isOutput=False)
    b1c2 = nc.declare_dram_parameter("b1c2", [128, 1], F32, isOutput=False)
    # 32 distinct narrow stationary slices (pair kk uses slice kk%32)
    stat_d = nc.declare_dram_parameter("stat", [128, 32, 64], F16, isOutput=False)
    out = nc.declare_dram_parameter("out", [R, K], F32, isOutput=True)

    with tile.TileContext(nc) as tc:
        with tc.tile_pool(name="singles", bufs=1) as singles:
            w1a_sb = singles.tile([D, 128], F16)
            w1b_sb = singles.tile([D, 128], F16)
            b1_sb = singles.tile([128, 1], F32)
            stat_sb = singles.tile([128, 32, 64], F16)
            zT = singles.tile([128, NT, 512], F16)
            zcT = singles.tile([128, R], F16)
            hjbT2 = singles.tile([128, K], F16)
            hibP = singles.tile([128, 2 * NPAIR], F32)

            # Each dma_start costs ~650ns of trigger time on its queue, so
            # split the input triggers across the sync and (idle) gpsimd
            # queues: sync feeds the hj chain (zT chunk 0 gates the first
            # silu), gpsimd feeds the bias path + stat.
            nc.sync.dma_start(out=zT[:, 0, :], in_=zTc_d[0, :, :])
            nc.sync.dma_start(out=w1b_sb[:], in_=w1b2[:])
            nc.sync.dma_start(out=b1_sb[:], in_=b1c2[:])
            for c in range(1, NT):
                nc.sync.dma_start(out=zT[:, c, :], in_=zTc_d[c, :, :])
            nc.gpsimd.dma_start(out=w1a_sb[:], in_=w1a2[:])
            nc.gpsimd.dma_start(out=zcT[:], in_=zcT_d[:])
            nc.gpsimd.dma_start(out=stat_sb[:], in_=stat_d[:])

            with tc.tile_pool(name="acc0p", bufs=1, space="PSUM") as acc0p:
                acc0 = acc0p.tile([128, NT, 512], F32, name="acc0")

                # ---- prologue: hi / hj projections (chunked) ----
                with tc.tile_pool(name="pp", bufs=1, space="PSUM") as pp:
                    # hiT (both halves) -> pair-bias columns; lane-aligned
                    # copies (even cols land on s=0 half, odd on s=1).
                    ph = pp.tile([128, R], F32, tag="ph")
                    nc.tensor.matmul(ph[:], w1a_sb[:], zcT[:], start=True, stop=True)
                    phr = ph.rearrange("e (k two) -> e two k", two=2)
                    nc.vector.tensor_copy(hibP[0:E, :], phr[0:E, 0, :])
                    nc.vector.tensor_copy(hibP[E:128, :], phr[E:128, 1, :])

                    for t in range(NT):
                        # hjT + b1, both (s,e) halves at once via [W1b|W1b].
                        pj = pp.tile([128, 512], F32, tag="pj", bufs=2)
                        nc.tensor.matmul(pj[:], w1b_sb[:], zT[:, t, :],
                                         start=True, stop=True)
                        nc.vector.tensor_scalar_add(
                            out=hjbT2[:, t * 512 : (t + 1) * 512],
                            in0=pj[:], scalar1=b1_sb[:],
                        )

                # ---- main loop: silu + e-contraction into PSUM ----
                with (
                    tc.tile_pool(name="acc1p", bufs=1, space="PSUM") as acc1p,
                    tc.tile_pool(name="hp", bufs=1) as hp,
                    tc.tile_pool(name="ep", bufs=1) as ep,
                    tc.tile_pool(name="sp", bufs=2) as sp,
                ):
                    acc1 = acc1p.tile([128, NT, 512], F32, name="acc1")
                    acc = {0: acc0, 1: acc1}

                    def contract(k, h_ap, t):
                        """acc rows 2kk,2kk+1 (64-aligned block) += W2 @ h"""
                        u, kk = divmod(k, NPAIR)
                        b, slot = divmod(kk, 32)
                        nc.tensor.matmul(
                            acc[u][64 * b : 64 * b + 64, t, :],
                            stat_sb[:, slot, :],
                            h_ap,
                            start=(slot == 0),
                            stop=(slot == 31),
                        )

                    # Warm-up pairs, chunked per 512-wide j tile: silu starts
                    # as soon as hjbT2 chunk 0 + hibP are ready (zT chunk
                    # DMAs and the pj chain are still in flight).
                    for k in range(WARM):
                        h = hp.tile([128, K], F16, tag="h", bufs=2)
                        for c in range(NT):
                            sl = slice(c * 512, (c + 1) * 512)
                            nc.scalar.activation(
                                out=h[:, sl], in_=hjbT2[:, sl], func=AF.Silu,
                                bias=hibP[:, k : k + 1], scale=1.0,
                            )
                            contract(k, h[:, sl], c)

                    # Steady state: DVE precomputes x = hjbT2 + bias for a
                    # group of pairs (fp16, 4x perf mode), then ONE wide
                    # ScalarE silu covers the group. First group is small
                    # (it only needs to bridge until the pipeline fills).
                    k0 = WARM
                    for gsz in [2] + [G] * ((R // 2 - WARM - 2 - TAIL) // G):
                        xg = hp.tile([128, G, K], F16, tag="xg", bufs=2)
                        hg = hp.tile([128, G, K], F16, tag="hg", bufs=2)
                        for g in range(gsz):
                            nc.vector.tensor_scalar_add(
                                out=xg[:, g, :], in0=hjbT2[:],
                                scalar1=hibP[:, k0 + g : k0 + g + 1],
                            )
                        nc.scalar.activation(
                            out=hg[:, 0:gsz, :].rearrange("p g j -> p (g j)"),
                            in_=xg[:, 0:gsz, :].rearrange("p g j -> p (g j)"),
                            func=AF.Silu,
                        )
                        for g in range(gsz):
                            for t in range(NT):
                                contract(k0 + g, hg[:, g, t * 512 : (t + 1) * 512], t)
                        k0 += gsz

                    # last pairs go per-pair so the final MM+softmax chain
                    # after the last silu is short
                    for k in range(k0, R // 2):
                        h = hp.tile([128, K], F16, tag="h", bufs=2)
                        nc.scalar.activation(
                            out=h[:], in_=hjbT2[:], func=AF.Silu,
                            bias=hibP[:, k : k + 1], scale=1.0,
                        )
                        for t in range(NT):
                            contract(k, h[:, t * 512 : (t + 1) * 512], t)

                    # ---- fused row softmax + store ----
                    # logits are O(+-6) here, so exp without max-subtraction
                    # is safe in fp32 and drops the serial max chain.
                    for u in range(R // 128):
                        tot = sp.tile([128, 1], F32, tag="tot")
                        rec = sp.tile([128, 1], F32, tag="rec")
                        ex = ep.tile([128, K], F32, tag=f"ex{u}")
                        nc.scalar.activation(
                            out=ex.rearrange("p (t j) -> p t j", t=NT),
                            in_=acc[u][:], func=AF.Exp,
                        )
                        nc.vector.tensor_reduce(
                            out=tot[:], in_=ex[:], axis=AX.X, op=ALU.add,
                        )
                        nc.vector.reciprocal(out=rec[:], in_=tot[:])
                        # chunked normalize+store so the DMA overlaps scale
                        for c in range(2):
                            sl = slice(c * (K // 2), (c + 1) * (K // 2))
                            nc.vector.tensor_scalar_mul(
                                out=ex[:, sl], in0=ex[:, sl], scalar1=rec[:]
                            )
                            nc.sync.dma_start(
                                out=out[u * 128 : (u + 1) * 128, sl],
                                in_=ex[:, sl],
                            )
    nc.finalize()  # Bacc.compile(): wait splitting, reg alloc, act tables
    return nc


_CACHE: dict = {}


def _get_nc() -> bass.Bass:
    if "nc" not in _CACHE:
        _CACHE["nc"] = build_nc()
    return _CACHE["nc"]


def make_in_maps(z, W1, b1, W2):
    z = np.ascontiguousarray(np.asarray(z, np.float32))
    W1 = np.asarray(W1, np.float32)
    b1 = np.asarray(b1, np.float32)
    W2 = np.asarray(W2, np.float32)

    # narrow stationary: pair kk uses slice kk%32, columns 2*(kk%32)+s
    stat = np.zeros((128, 32, 64), np.float32)
    w2col = W2[:, 0]
    for slot in range(32):
        for s in range(2):
            stat[s * E : (s + 1) * E, slot, 2 * slot + s] = w2col
    stat = stat.astype(np.float16)
    b1c2 = np.ascontiguousarray(np.tile(b1, 2).reshape(128, 1))
    w1a2 = np.ascontiguousarray(np.tile(W1[:D], (1, 2)).astype(np.float16))
    w1b2 = np.ascontiguousarray(np.tile(W1[D:], (1, 2)).astype(np.float16))
    zT16 = np.ascontiguousarray(z.astype(np.float16).T)  # (D, K)
    zTc = np.ascontiguousarray(
        zT16.reshape(D, NT, 512).transpose(1, 0, 2)
    )  # (NT, D, 512)

    in_maps = []
    for c in range(NCORES):
        in_maps.append(
            {
                "zTc": zTc,
                "zcT": np.ascontiguousarray(zT16[:, c * R : (c + 1) * R]),
                "w1a2": w1a2,
                "w1b2": w1b2,
                "b1c2": b1c2,
                "stat": stat,
            }
        )
    return in_maps


def run(inputs: dict, trace: bool = False):
    """Run the bass kernel; returns (full_output, BassKernelResults)."""
    nc = _get_nc()
    in_maps = make_in_maps(inputs["z"], inputs["W1"], inputs["b1"], inputs["W2"])
    res = run_bass_kernel_spmd(nc, in_maps, list(range(NCORES)), trace=trace)
    full = np.concatenate([res.results[c]["out"] for c in range(NCORES)], axis=0)
    return full, res


def kernel(**inputs) -> np.ndarray:
    full, _ = run(inputs, trace=False)
    return full


# revision 17
# speedup vs baseline: 1.8571x; 1.2049x over previous
"""Trainium2 Bass kernel for nn_DeterministicAdjacency (gnn_message_passing).

Math (reference):
    hi = z @ W1[:D]            # (K, E)
    hj = z @ W1[D:]            # (K, E)
    h  = silu(hi[:,None,:] + hj[None,:,:] + b1)    # (K, K, E)
    logits = einsum('ije,eo->ij', h, W2) + b2      # (K, K)
    out = softmax(logits, axis=-1)

b2 is dropped: softmax is invariant to a constant shift.

Sharding: rows (i / query dim) split across 8 cores, 256 rows each. Each core
computes its 256 rows of logits against the full z and does local row softmax.

Per-core layout ("layout A", e on partitions):
  - hjbT2 (128p=(s,e), 2048f=j): hj^T + b1, duplicated on both partition
    halves (s = row-parity slot), fp16. Computed in 4 j-chunks of 512 so the
    first silu can start while the zT DMA is still landing.
  - hibP (128p=(s,e), 128f=k): bias columns; column k holds
    [hi[2k,:] ; hi[2k+1,:]] so one ScalarE activation instruction computes
    silu for TWO query rows x all 2048 keys x all 64 features.
  - contraction over e via TensorE with a NARROW stationary: pairs are
    processed in blocks of 16; pair kk uses a (128 x 32) stationary slice
    (stat[(s,e), 2*(kk%16)+s] = W2[e]) and accumulates into the 32-aligned
    psum partition slice [32*(kk//16) : +32] of a (128, 4, 512) accumulator.
    Only 32 of 128 PE columns are active -> less PE/SBUF energy (this kernel
    is ACT-bound and the chip duty-throttles on power: ~0.8 avg util limit
    was observed with the dense 128-wide stationary layout).
  - h/xg/hjbT2 are fp16: DVE runs in 4x perf mode for the bias-add
    precompute, halving SBUF traffic; PE fp16 path is 1 cyc/row.
  - steady state: DVE precomputes x = hjbT2 + bias for groups of 8 pairs,
    then ONE 16384-wide ScalarE silu amortizes the per-instruction bubble.
  - softmax fused on the PSUM accumulators (ACT exp, DVE row-sum +
    reciprocal + scale; logits are O(+-6) so max-subtraction is skipped),
    then chunked DMA out. Row sums use DVE tensor_reduce instead of the ACT
    accumulator: READ_ACCUMULATOR forces a serializing engine drain between
    the two exp instructions.
"""

import numpy as np

import concourse.bass as bass
import concourse.bacc as bacc
import concourse.mybir as mybir
from concourse import tile
from concourse.bass_utils import run_bass_kernel_spmd

K, D, E = 2048, 128, 64
NCORES = 8
R = K // NCORES            # 256 rows per core
NPAIR = 64                 # row pairs per 128-row i-tile
NT = 4                     # 512-wide j tiles
WARM = 2                   # chunked warm-up pairs
G = 8                      # steady-state group size
TAIL = 4                   # per-pair tail pairs
F32 = mybir.dt.float32
F16 = mybir.dt.float16
AF = mybir.ActivationFunctionType
AX = mybir.AxisListType
ALU = mybir.AluOpType


def build_nc() -> bass.Bass:
    # Bacc (not raw Bass): its finalize() runs generate_event_semaphores(),
    # which splits multi-sem waits — TRN2 instructions hold at most one wait.
    nc = bacc.Bacc(None, target_bir_lowering=False)
    # zTc comes in fp16, pre-transposed and pre-chunked (host layout prep):
    # contiguous 128KB DMAs, d already on partitions for the hj contraction.
    zTc_d = nc.declare_dram_parameter("zTc", [NT, D, 512], F16, isOutput=False)
    zcT_d = nc.declare_dram_parameter("zcT", [D, R], F16, isOutput=False)
    # w1a2/w1b2 = [W1a | W1a], [W1b | W1b]: one matmul emits both
    # partition-halves of the (s,e)-duplicated layouts directly.
    w1a2 = nc.declare_dram_parameter("w1a2", [D, 128], F16, isOutput=False)
    w1b2 = nc.declare_dram_parameter("w1b2", [D, 128], F16, isOutput=False)
    b1c2 = nc.declare_dram_parameter("b1c2", [128, 1], F32, isOutput=False)
    # 32 distinct narrow stationary slices (pair kk uses slice kk%32)
    stat_d = nc.declare_dram_parameter("stat", [128, 32, 64], F16, isOutput=False)
    out = nc.declare_dram_parameter("out", [R, K], F32, isOutput=True)

    with tile.TileContext(nc) as tc:
        with tc.tile_pool(name="singles", bufs=1) as singles:
            w1a_sb = singles.tile([D, 128], F16)
            w1b_sb = singles.tile([D, 128], F16)
            b1_sb = singles.tile([128, 1], F32)
            stat_sb = singles.tile([128, 32, 64], F16)
            zT = singles.tile([128, NT, 512], F16)
            zcT = singles.tile([128, R], F16)
            hjbT2 = singles.tile([128, K], F16)
            hibP = singles.tile([128, 2 * NPAIR], F32)

            # Each dma_start costs ~650ns of trigger time on its queue, so
            # order the sync-queue triggers by when the data gates compute
            # (zT chunk 0 + w1b + b1 gate the hj chain; w1a + zcT gate the
            # bias path) and push the late-needed bulk (stat, zT tail
            # chunks) onto the idle gpsimd queue's software DGE.
            nc.sync.dma_start(out=zT[:, 0, :], in_=zTc_d[0, :, :])
            nc.sync.dma_start(out=w1b_sb[:], in_=w1b2[:])
            nc.sync.dma_start(out=b1_sb[:], in_=b1c2[:])
            nc.sync.dma_start(out=w1a_sb[:], in_=w1a2[:])
            nc.sync.dma_start(out=zcT[:], in_=zcT_d[:])
            for c in range(1, NT):
                nc.sync.dma_start(out=zT[:, c, :], in_=zTc_d[c, :, :])
            nc.gpsimd.dma_start(out=stat_sb[:], in_=stat_d[:])

            with tc.tile_pool(name="acc0p", bufs=1, space="PSUM") as acc0p:
                acc0 = acc0p.tile([128, NT, 512], F32, name="acc0")

                # ---- prologue: hi / hj projections (chunked) ----
                with tc.tile_pool(name="pp", bufs=1, space="PSUM") as pp:
                    # hiT (both halves) -> pair-bias columns; lane-aligned
                    # copies (even cols land on s=0 half, odd on s=1).
                    ph = pp.tile([128, R], F32, tag="ph")
                    nc.tensor.matmul(ph[:], w1a_sb[:], zcT[:], start=True, stop=True)
                    phr = ph.rearrange("e (k two) -> e two k", two=2)
                    nc.vector.tensor_copy(hibP[0:E, :], phr[0:E, 0, :])
                    nc.vector.tensor_copy(hibP[E:128, :], phr[E:128, 1, :])

                    for t in range(NT):
                        # hjT + b1, both (s,e) halves at once via [W1b|W1b].
                        pj = pp.tile([128, 512], F32, tag="pj", bufs=2)
                        nc.tensor.matmul(pj[:], w1b_sb[:], zT[:, t, :],
                                         start=True, stop=True)
                        nc.vector.tensor_scalar_add(
                            out=hjbT2[:, t * 512 : (t + 1) * 512],
                            in0=pj[:], scalar1=b1_sb[:],
                        )

                # ---- main loop: silu + e-contraction into PSUM ----
                with (
                    tc.tile_pool(name="acc1p", bufs=1, space="PSUM") as acc1p,
                    tc.tile_pool(name="hp", bufs=1) as hp,
                    tc.tile_pool(name="ep", bufs=1) as ep,
                    tc.tile_pool(name="sp", bufs=2) as sp,
                ):
                    acc1 = acc1p.tile([128, NT, 512], F32, name="acc1")
                    acc = {0: acc0, 1: acc1}

                    def contract(k, h_ap, t):
                        """acc rows 2kk,2kk+1 (64-aligned block) += W2 @ h"""
                        u, kk = divmod(k, NPAIR)
                        b, slot = divmod(kk, 32)
                        nc.tensor.matmul(
                            acc[u][64 * b : 64 * b + 64, t, :],
                            stat_sb[:, slot, :],
                            h_ap,
                            start=(slot == 0),
                            stop=(slot == 31),
                        )

                    # Warm-up pairs, chunked per 512-wide j tile: silu starts
                    # as soon as hjbT2 chunk 0 + hibP are ready (zT chunk
                    # DMAs and the pj chain are still in flight).
                    for k in range(WARM):
                        h = hp.tile([128, K], F16, tag="h", bufs=2)
                        for c in range(NT):
                            sl = slice(c * 512, (c + 1) * 512)
                            nc.scalar.activation(
                                out=h[:, sl], in_=hjbT2[:, sl], func=AF.Silu,
                                bias=hibP[:, k : k + 1], scale=1.0,
                            )
                            contract(k, h[:, sl], c)

                    # Steady state: DVE precomputes x = hjbT2 + bias for a
                    # group of pairs (fp16, 4x perf mode), then ONE wide
                    # ScalarE silu covers the group. First group is small
                    # (it only needs to bridge until the pipeline fills).
                    k0 = WARM
                    for gsz in [2] + [G] * ((R // 2 - WARM - 2 - TAIL) // G):
                        xg = hp.tile([128, G, K], F16, tag="xg", bufs=2)
                        hg = hp.tile([128, G, K], F16, tag="hg", bufs=2)
                        for g in range(gsz):
                            nc.vector.tensor_scalar_add(
                                out=xg[:, g, :], in0=hjbT2[:],
                                scalar1=hibP[:, k0 + g : k0 + g + 1],
                            )
                        nc.scalar.activation(
                            out=hg[:, 0:gsz, :].rearrange("p g j -> p (g j)"),
                            in_=xg[:, 0:gsz, :].rearrange("p g j -> p (g j)"),
                            func=AF.Silu,
                        )
                        for g in range(gsz):
                            for t in range(NT):
                                contract(k0 + g, hg[:, g, t * 512 : (t + 1) * 512], t)
                        k0 += gsz

                    # last pairs go per-pair so the final MM+softmax chain
                    # after the last silu is short
                    for k in range(k0, R // 2):
                        h = hp.tile([128, K], F16, tag="h", bufs=2)
                        nc.scalar.activation(
                            out=h[:], in_=hjbT2[:], func=AF.Silu,
                            bias=hibP[:, k : k + 1], scale=1.0,
                        )
                        for t in range(NT):
                            contract(k, h[:, t * 512 : (t + 1) * 512], t)

                    # ---- fused row softmax + store ----
                    # logits are O(+-6) here, so exp without max-subtraction
                    # is safe in fp32 and drops the serial max chain.
                    for u in range(R // 128):
                        tot = sp.tile([128, 1], F32, tag="tot")
                        rec = sp.tile([128, 1], F32, tag="rec")
                        ex = ep.tile([128, K], F32, tag=f"ex{u}")
                        nc.scalar.activation(
                            out=ex.rearrange("p (t j) -> p t j", t=NT),
                            in_=acc[u][:], func=AF.Exp,
                        )
                        nc.vector.tensor_reduce(
                            out=tot[:], in_=ex[:], axis=AX.X, op=ALU.add,
                        )
                        nc.vector.reciprocal(out=rec[:], in_=tot[:])
                        # chunked normalize+store so the DMA overlaps scale
                        for c in range(2):
                            sl = slice(c * (K // 2), (c + 1) * (K // 2))
                            nc.vector.tensor_scalar_mul(
                                out=ex[:, sl], in0=ex[:, sl], scalar1=rec[:]
                            )
                            nc.sync.dma_start(
                                out=out[u * 128 : (u + 1) * 128, sl],
                                in_=ex[:, sl],
                            )
    nc.finalize()  # Bacc.compile(): wait splitting, reg alloc, act tables
    return nc


_CACHE: dict = {}


def _get_nc() -> bass.Bass:
    if "nc" not in _CACHE:
        _CACHE["nc"] = build_nc()
    return _CACHE["nc"]


def make_in_maps(z, W1, b1, W2):
    z = np.ascontiguousarray(np.asarray(z, np.float32))
    W1 = np.asarray(W1, np.float32)
    b1 = np.asarray(b1, np.float32)
    W2 = np.asarray(W2, np.float32)

    # narrow stationary: pair kk uses slice kk%32, columns 2*(kk%32)+s
    stat = np.zeros((128, 32, 64), np.float32)
    w2col = W2[:, 0]
    for slot in range(32):
        for s in range(2):
            stat[s * E : (s + 1) * E, slot, 2 * slot + s] = w2col
    stat = stat.astype(np.float16)
    b1c2 = np.ascontiguousarray(np.tile(b1, 2).reshape(128, 1))
    w1a2 = np.ascontiguousarray(np.tile(W1[:D], (1, 2)).astype(np.float16))
    w1b2 = np.ascontiguousarray(np.tile(W1[D:], (1, 2)).astype(np.float16))
    zT16 = np.ascontiguousarray(z.astype(np.float16).T)  # (D, K)
    zTc = np.ascontiguousarray(
        zT16.reshape(D, NT, 512).transpose(1, 0, 2)
    )  # (NT, D, 512)

    in_maps = []
    for c in range(NCORES):
        in_maps.append(
            {
                "zTc": zTc,
                "zcT": np.ascontiguousarray(zT16[:, c * R : (c + 1) * R]),
                "w1a2": w1a2,
                "w1b2": w1b2,
                "b1c2": b1c2,
                "stat": stat,
            }
        )
    return in_maps


def run(inputs: dict, trace: bool = False):
    """Run the bass kernel; returns (full_output, BassKernelResults)."""
    nc = _get_nc()
    in_maps = make_in_maps(inputs["z"], inputs["W1"], inputs["b1"], inputs["W2"])
    res = run_bass_kernel_spmd(nc, in_maps, list(range(NCORES)), trace=trace)
    full = np.concatenate([res.results[c]["out"] for c in range(NCORES)], axis=0)
    return full, res


def kernel(**inputs) -> np.ndarray:
    full, _ = run(inputs, trace=False)
    return full


# revision 20
# speedup vs baseline: 1.8909x; 1.0182x over previous
"""Trainium2 Bass kernel for nn_DeterministicAdjacency (gnn_message_passing).

Math (reference):
    hi = z @ W1[:D]            # (K, E)
    hj = z @ W1[D:]            # (K, E)
    h  = silu(hi[:,None,:] + hj[None,:,:] + b1)    # (K, K, E)
    logits = einsum('ije,eo->ij', h, W2) + b2      # (K, K)
    out = softmax(logits, axis=-1)

b2 is dropped: softmax is invariant to a constant shift.

Sharding: rows (i / query dim) split across 8 cores, 256 rows each. Each core
computes its 256 rows of logits against the full z and does local row softmax.

Per-core layout ("layout A", e on partitions):
  - hjbT2 (128p=(s,e), 2048f=j): hj^T + b1, duplicated on both partition
    halves (s = row-parity slot), fp16. Computed in 4 j-chunks of 512 so the
    first silu can start while the zT DMA is still landing.
  - hibP (128p=(s,e), 128f=k): bias columns; column k holds
    [hi[2k,:] ; hi[2k+1,:]] so one ScalarE activation instruction computes
    silu for TWO query rows x all 2048 keys x all 64 features.
  - contraction over e via TensorE with a NARROW stationary: pairs are
    processed in blocks of 16; pair kk uses a (128 x 32) stationary slice
    (stat[(s,e), 2*(kk%16)+s] = W2[e]) and accumulates into the 32-aligned
    psum partition slice [32*(kk//16) : +32] of a (128, 4, 512) accumulator.
    Only 32 of 128 PE columns are active -> less PE/SBUF energy (this kernel
    is ACT-bound and the chip duty-throttles on power: ~0.8 avg util limit
    was observed with the dense 128-wide stationary layout).
  - h/xg/hjbT2 are fp16: DVE runs in 4x perf mode for the bias-add
    precompute, halving SBUF traffic; PE fp16 path is 1 cyc/row.
  - steady state: DVE precomputes x = hjbT2 + bias for groups of 8 pairs,
    then ONE 16384-wide ScalarE silu amortizes the per-instruction bubble.
  - softmax fused on the PSUM accumulators (ACT exp, DVE row-sum +
    reciprocal + scale; logits are O(+-6) so max-subtraction is skipped),
    then chunked DMA out. Row sums use DVE tensor_reduce instead of the ACT
    accumulator: READ_ACCUMULATOR forces a serializing engine drain between
    the two exp instructions.
"""

import numpy as np

import concourse.bass as bass
import concourse.bacc as bacc
import concourse.mybir as mybir
from concourse import tile
from concourse.bass_utils import run_bass_kernel_spmd

K, D, E = 2048, 128, 64
NCORES = 8
R = K // NCORES            # 256 rows per core
NPAIR = 64                 # row pairs per 128-row i-tile
NT = 4                     # 512-wide j tiles
WARM = 2                   # chunked warm-up pairs
G = 8                      # steady-state group size
F32 = mybir.dt.float32
F16 = mybir.dt.float16
AF = mybir.ActivationFunctionType
AX = mybir.AxisListType
ALU = mybir.AluOpType


def build_nc() -> bass.Bass:
    # Bacc (not raw Bass): its finalize() runs generate_event_semaphores(),
    # which splits multi-sem waits — TRN2 instructions hold at most one wait.
    nc = bacc.Bacc(None, target_bir_lowering=False)
    # zTc comes in fp16, pre-transposed and pre-chunked (host layout prep):
    # contiguous 128KB DMAs, d already on partitions for the hj contraction.
    zTc_d = nc.declare_dram_parameter("zTc", [NT, D, 512], F16, isOutput=False)
    zcT_d = nc.declare_dram_parameter("zcT", [D, R], F16, isOutput=False)
    # w1a2/w1b2 = [W1a | W1a], [W1b | W1b]: one matmul emits both
    # partition-halves of the (s,e)-duplicated layouts directly.
    w1a2 = nc.declare_dram_parameter("w1a2", [D, 128], F16, isOutput=False)
    w1b2 = nc.declare_dram_parameter("w1b2", [D, 128], F16, isOutput=False)
    b1c2 = nc.declare_dram_parameter("b1c2", [128, 1], F32, isOutput=False)
    # 32 distinct narrow stationary slices (pair kk uses slice kk%32)
    stat_d = nc.declare_dram_parameter("stat", [128, 32, 64], F16, isOutput=False)
    out = nc.declare_dram_parameter("out", [R, K], F32, isOutput=True)

    with tile.TileContext(nc) as tc:
        with tc.tile_pool(name="singles", bufs=1) as singles:
            w1a_sb = singles.tile([D, 128], F16)
            w1b_sb = singles.tile([D, 128], F16)
            b1_sb = singles.tile([128, 1], F32)
            stat_sb = singles.tile([128, 32, 64], F16)
            zT = singles.tile([128, NT, 512], F16)
            zcT = singles.tile([128, R], F16)
            hjbT2 = singles.tile([128, K], F16)
            hibP = singles.tile([128, 2 * NPAIR], F32)

            # Each dma_start costs ~650ns of trigger time on its queue, so
            # order the sync-queue triggers by when the data gates compute
            # (zT chunk 0 + w1b + b1 gate the hj chain; w1a + zcT gate the
            # bias path) and push the late-needed bulk (stat, zT tail
            # chunks) onto the idle gpsimd queue's software DGE.
            nc.sync.dma_start(out=zT[:, 0, :], in_=zTc_d[0, :, :])
            nc.sync.dma_start(out=w1b_sb[:], in_=w1b2[:])
            nc.sync.dma_start(out=b1_sb[:], in_=b1c2[:])
            nc.sync.dma_start(out=w1a_sb[:], in_=w1a2[:])
            nc.sync.dma_start(out=zcT[:], in_=zcT_d[:])
            for c in range(1, NT):
                nc.sync.dma_start(out=zT[:, c, :], in_=zTc_d[c, :, :])
            nc.gpsimd.dma_start(out=stat_sb[:], in_=stat_d[:])

            with tc.tile_pool(name="acc0p", bufs=1, space="PSUM") as acc0p:
                acc0 = acc0p.tile([128, NT, 512], F32, name="acc0")

                # ---- prologue: hi / hj projections (chunked) ----
                with tc.tile_pool(name="pp", bufs=1, space="PSUM") as pp:
                    # hiT (both halves) -> pair-bias columns; lane-aligned
                    # copies (even cols land on s=0 half, odd on s=1).
                    ph = pp.tile([128, R], F32, tag="ph")
                    nc.tensor.matmul(ph[:], w1a_sb[:], zcT[:], start=True, stop=True)
                    phr = ph.rearrange("e (k two) -> e two k", two=2)
                    nc.vector.tensor_copy(hibP[0:E, :], phr[0:E, 0, :])
                    nc.vector.tensor_copy(hibP[E:128, :], phr[E:128, 1, :])

                    for t in range(NT):
                        # hjT + b1, both (s,e) halves at once via [W1b|W1b].
                        pj = pp.tile([128, 512], F32, tag="pj", bufs=2)
                        nc.tensor.matmul(pj[:], w1b_sb[:], zT[:, t, :],
                                         start=True, stop=True)
                        nc.vector.tensor_scalar_add(
                            out=hjbT2[:, t * 512 : (t + 1) * 512],
                            in0=pj[:], scalar1=b1_sb[:],
                        )

                # ---- main loop: silu + e-contraction into PSUM ----
                with (
                    tc.tile_pool(name="acc1p", bufs=1, space="PSUM") as acc1p,
                    tc.tile_pool(name="hp", bufs=1) as hp,
                    tc.tile_pool(name="ep", bufs=1) as ep,
                    tc.tile_pool(name="sp", bufs=2) as sp,
                ):
                    acc1 = acc1p.tile([128, NT, 512], F32, name="acc1")
                    acc = {0: acc0, 1: acc1}

                    def contract(k, h_ap, t):
                        """acc rows 2kk,2kk+1 (64-aligned block) += W2 @ h"""
                        u, kk = divmod(k, NPAIR)
                        b, slot = divmod(kk, 32)
                        nc.tensor.matmul(
                            acc[u][64 * b : 64 * b + 64, t, :],
                            stat_sb[:, slot, :],
                            h_ap,
                            start=(slot == 0),
                            stop=(slot == 31),
                        )

                    # Warm-up pairs, chunked per 512-wide j tile: silu starts
                    # as soon as hjbT2 chunk 0 + hibP are ready (zT chunk
                    # DMAs and the pj chain are still in flight).
                    for k in range(WARM):
                        h = hp.tile([128, K], F16, tag="h", bufs=2)
                        for c in range(NT):
                            sl = slice(c * 512, (c + 1) * 512)
                            nc.scalar.activation(
                                out=h[:, sl], in_=hjbT2[:, sl], func=AF.Silu,
                                bias=hibP[:, k : k + 1], scale=1.0,
                            )
                            contract(k, h[:, sl], c)

                    # Steady state: DVE precomputes x = hjbT2 + bias for a
                    # group of pairs (fp16, 4x perf mode), then ONE wide
                    # ScalarE silu covers the group. First group is small
                    # (it only needs to bridge until the pipeline fills).
                    # group-size schedule: bridge, steady G=8, then taper so
                    # the PE drain after the last silu is one pair, not a
                    # whole group (exp(u1) waits on the final matmul).
                    k0 = WARM
                    for gsz in [2] + [G] * 14 + [4, 4, 2]:
                        xg = hp.tile([128, G, K], F16, tag="xg", bufs=2)
                        hg = hp.tile([128, G, K], F16, tag="hg", bufs=2)
                        for g in range(gsz):
                            nc.vector.tensor_scalar_add(
                                out=xg[:, g, :], in0=hjbT2[:],
                                scalar1=hibP[:, k0 + g : k0 + g + 1],
                            )
                        nc.scalar.activation(
                            out=hg[:, 0:gsz, :].rearrange("p g j -> p (g j)"),
                            in_=xg[:, 0:gsz, :].rearrange("p g j -> p (g j)"),
                            func=AF.Silu,
                        )
                        for g in range(gsz):
                            for t in range(NT):
                                contract(k0 + g, hg[:, g, t * 512 : (t + 1) * 512], t)
                        k0 += gsz

                    # last pairs go per-pair so the final MM+softmax chain
                    # after the last silu is short
                    for k in range(k0, R // 2):
                        h = hp.tile([128, K], F16, tag="h", bufs=2)
                        nc.scalar.activation(
                            out=h[:], in_=hjbT2[:], func=AF.Silu,
                            bias=hibP[:, k : k + 1], scale=1.0,
                        )
                        for t in range(NT):
                            contract(k, h[:, t * 512 : (t + 1) * 512], t)

                    # ---- fused row softmax + store ----
                    # logits are O(+-6) here, so exp without max-subtraction
                    # is safe in fp32 and drops the serial max chain.
                    for u in range(R // 128):
                        tot = sp.tile([128, 1], F32, tag="tot")
                        rec = sp.tile([128, 1], F32, tag="rec")
                        ex = ep.tile([128, K], F32, tag=f"ex{u}")
                        if u == 0:
                            # u0's softmax overlaps the remaining silu
                            # stream; DVE row-sum keeps the ACT queue free.
                            nc.scalar.activation(
                                out=ex.rearrange("p (t j) -> p t j", t=NT),
                                in_=acc[u][:], func=AF.Exp,
                            )
                            nc.vector.tensor_reduce(
                                out=tot[:], in_=ex[:], axis=AX.X, op=ALU.add,
                            )
                        else:
                            # u1 is the tail critical path: the fused ACT
                            # accumulator is ~2us faster than a DVE reduce.
                            nc.scalar.activation(
                                out=ex.rearrange("p (t j) -> p t j", t=NT),
                                in_=acc[u][:], func=AF.Exp,
                                accum_out=tot[:],
                            )
                        nc.vector.reciprocal(out=rec[:], in_=tot[:])
                        # chunked normalize+store so the DMA overlaps scale
                        for c in range(2):
                            sl = slice(c * (K // 2), (c + 1) * (K // 2))
                            nc.vector.tensor_scalar_mul(
                                out=ex[:, sl], in0=ex[:, sl], scalar1=rec[:]
                            )
                            nc.sync.dma_start(
                                out=out[u * 128 : (u + 1) * 128, sl],
                                in_=ex[:, sl],
                            )
    nc.finalize()  # Bacc.compile(): wait splitting, reg alloc, act tables
    return nc


_CACHE: dict = {}


def _get_nc() -> bass.Bass:
    if "nc" not in _CACHE:
        _CACHE["nc"] = build_nc()
    return _CACHE["nc"]


def make_in_maps(z, W1, b1, W2):
    z = np.ascontiguousarray(np.asarray(z, np.float32))
    W1 = np.asarray(W1, np.float32)
    b1 = np.asarray(b1, np.float32)
    W2 = np.asarray(W2, np.float32)

    # narrow stationary: pair kk uses slice kk%32, columns 2*(kk%32)+s
    stat = np.zeros((128, 32, 64), np.float32)
    w2col = W2[:, 0]
    for slot in range(32):
        for s in range(2):
            stat[s * E : (s + 1) * E, slot, 2 * slot + s] = w2col
    stat = stat.astype(np.float16)
    b1c2 = np.ascontiguousarray(np.tile(b1, 2).reshape(128, 1))
    w1a2 = np.ascontiguousarray(np.tile(W1[:D], (1, 2)).astype(np.float16))
    w1b2 = np.ascontiguousarray(np.tile(W1[D:], (1, 2)).astype(np.float16))
    zT16 = np.ascontiguousarray(z.astype(np.float16).T)  # (D, K)
    zTc = np.ascontiguousarray(
        zT16.reshape(D, NT, 512).transpose(1, 0, 2)
    )  # (NT, D, 512)

    in_maps = []
    for c in range(NCORES):
        in_maps.append(
            {
                "zTc": zTc,
                "zcT": np.ascontiguousarray(zT16[:, c * R : (c + 1) * R]),
                "w1a2": w1a2,
                "w1b2": w1b2,
                "b1c2": b1c2,
                "stat": stat,
            }
        )
    return in_maps


def run(inputs: dict, trace: bool = False):
    """Run the bass kernel; returns (full_output, BassKernelResults)."""
    nc = _get_nc()
    in_maps = make_in_maps(inputs["z"], inputs["W1"], inputs["b1"], inputs["W2"])
    res = run_bass_kernel_spmd(nc, in_maps, list(range(NCORES)), trace=trace)
    full = np.concatenate([res.results[c]["out"] for c in range(NCORES)], axis=0)
    return full, res


def kernel(**inputs) -> np.ndarray:
    full, _ = run(inputs, trace=False)
    return full
